# revision 40
# baseline (speedup 1.0000x reference)
"""Trainium2 Bass kernel for Attention4DDownsample (EfficientFormer-style).

Strategy: pure data parallelism over batch (256 -> 32 examples/core x 8 cores).
All BN scales/biases + attention scale folded into conv weights on host.
fp16 on the PE; fp32 PSUM accumulation. x and v4 carry a zero halo (16x16
spatial) so depthwise taps are full-rect matmuls; the q-path depthwise conv is
folded into the q 1x1 projection (10 fused weight sets).

v3 attention datapath (per example):
 - S for a head pair = ONE matmul: contraction rows 32p:32p+32 hold both
   heads' q (lhsT, zero-interleaved 64-col blocks) and k (rhs = k_sb rows
   directly, PE quadrant tile_position=(32p,0)) -> (128, 256) PSUM region
   (even head rows 0:64, odd rows 64:128). No per-head shift DMAs.
 - The relative-position bias lands first via one (128,512) matmul from a
   prebuilt bias tile (iden lhsT), zeroing pad rows; halo key cols get -60000.
 - Exp+accum and 1/den renorm run at full 128-partition width, 4 ops each.
 - A^T and V^T are produced by single SBUF->SBUF XBAR DMA transposes
   ((128,1024) -> (128, 8, 128) blocked), freeing PE/DVE/PSUM entirely.
 - AV + v_local depthwise accumulate in one PSUM bank per 128-channel group.
"""

import sys

sys.path.insert(0, "/opt/trn_rl_repo")

import numpy as np

B, C, H, W = 256, 384, 14, 14
HEADS, KD, D = 8, 16, 64
NHKD, DH = 128, 512
OUT = 384
N2, N = 49, 196
HP, WP = 16, 16  # padded spatial
NP = HP * WP  # 256
NCORES = 8
EPC = B // NCORES  # 32 examples per core
SG = 8  # examples per super-group

TAPS = [(dy, dx) for dy in range(3) for dx in range(3)]
TAPS_C = [(1, 1)] + [t for t in TAPS if t != (1, 1)]  # center first (start=True)


def build_bass(epc=EPC, reps=1, at_depth=None, hoist_vt=None, use_xbar=None, q2p_pool=False, y_eng='sp', bias_dma=False):
    import os

    import concourse.bass as bass
    import concourse.tile as tile
    from concourse import bacc, mybir

    if use_xbar is None:
        use_xbar = os.environ.get("BASSK_NO_XBAR", "0") != "1"
    use_prefetch = os.environ.get("BASSK_NO_PREFETCH", "0") != "1"
    if at_depth is None:
        at_depth = int(os.environ.get("BASSK_AT_DEPTH", "2"))
    if hoist_vt is None:
        hoist_vt = os.environ.get("BASSK_HOIST_VT", "1") != "0"

    f16 = mybir.dt.float16
    f32 = mybir.dt.float32
    AF = mybir.ActivationFunctionType

    nsg = epc // SG
    nc = bacc.Bacc(trn_type="TRN2", debug=False)

    # ---- DRAM I/O ----
    x_t = nc.dram_tensor("x", (3, 128, epc, NP), f16, kind="ExternalInput")
    qw_t = nc.dram_tensor("qw", (128, 3, 10, 128), f16, kind="ExternalInput")
    kw_t = nc.dram_tensor("kw", (128, 3, 128), f16, kind="ExternalInput")
    vw_t = nc.dram_tensor("vw", (128, 3, 4, 128), f16, kind="ExternalInput")
    pw_t = nc.dram_tensor("pw", (128, 4, 3, 128), f16, kind="ExternalInput")
    vld_t = nc.dram_tensor("vld", (128, 4, 9, 128), f16, kind="ExternalInput")
    # biasp[r, 512j + 256b + kk]: bias for pair p=2j+b, q-row r, key kk
    biasp_t = nc.dram_tensor("biasp", (128, 1024), f16, kind="ExternalInput")
    if bias_dma:
        biasp32_t = nc.dram_tensor("biasp32", (128, 1024), f32, kind="ExternalInput")
    iden_t = nc.dram_tensor("iden", (128, 128), f16, kind="ExternalInput")
    # bvec cols: vb[0:4] pb[4:7] vlb[7:11] kb[11] qb[12]
    bvec_t = nc.dram_tensor("bvec", (128, 13), f32, kind="ExternalInput")
    y_t = nc.dram_tensor("y", (3, 128, epc, 49), f32, kind="ExternalOutput")

    with tile.TileContext(nc) as tc:
        with (
            tc.tile_pool(name="consts", bufs=1) as consts,
            tc.tile_pool(name="xp", bufs=2) as xp,
            tc.tile_pool(name="sb_small", bufs=2) as sbs,
            tc.tile_pool(name="sb_kv", bufs=2) as sbkv,
            tc.tile_pool(name="sb_vt", bufs=2) as sbvt,
            tc.tile_pool(name="sb_a", bufs=3) as sba,
            tc.tile_pool(name="sb_at", bufs=2) as sbat,
            tc.tile_pool(name="sb_z", bufs=2) as sbz,
            tc.tile_pool(name="sb_y", bufs=2) as sby,
            tc.tile_pool(name="psC", bufs=2, space="PSUM") as psC,
            tc.tile_pool(name="psS", bufs=2 if use_xbar else 1, space="PSUM") as psS,
            tc.tile_pool(name="psVL", bufs=2 if use_xbar else 1, space="PSUM") as psVL,
            tc.tile_pool(name="psT", bufs=1, space="PSUM") as psT,
        ):
            # ---- load constants ----
            qw_sb = consts.tile([128, 3 * 10 * 128], f16, name="qw_sb")
            nc.sync.dma_start(out=qw_sb, in_=qw_t.ap().rearrange("p a b c -> p (a b c)"))
            kw_sb = consts.tile([128, 3 * 128], f16, name="kw_sb")
            nc.sync.dma_start(out=kw_sb, in_=kw_t.ap().rearrange("p a b -> p (a b)"))
            vw_sb = consts.tile([128, 3 * 4 * 128], f16, name="vw_sb")
            nc.sync.dma_start(out=vw_sb, in_=vw_t.ap().rearrange("p a b c -> p (a b c)"))
            pw_sb = consts.tile([128, 4 * 3 * 128], f16, name="pw_sb")
            nc.sync.dma_start(out=pw_sb, in_=pw_t.ap().rearrange("p a b c -> p (a b c)"))
            vld_sb = consts.tile([128, 4 * 9 * 128], f16, name="vld_sb")
            nc.sync.dma_start(out=vld_sb, in_=vld_t.ap().rearrange("p a b c -> p (a b c)"))
            iden_sb = consts.tile([128, 128], f16, name="iden_sb")
            nc.sync.dma_start(out=iden_sb, in_=iden_t.ap())
            biasp_sb = consts.tile([128, 1024], f16, name="biasp_sb")
            nc.sync.dma_start(out=biasp_sb, in_=biasp_t.ap())
            if bias_dma:
                biasp32_sb = consts.tile([128, 1024], f32, name="biasp32_sb")
                nc.sync.dma_start(out=biasp32_sb, in_=biasp32_t.ap())
            bvec_sb = consts.tile([128, 13], f32, name="bvec_sb")
            nc.sync.dma_start(out=bvec_sb, in_=bvec_t.ap())

            qw_l = lambda kc, t: qw_sb[:, (kc * 10 + t) * 128:][:, :128]
            kw_l = lambda kc: kw_sb[:, kc * 128:][:, :128]
            vw_l = lambda kc, m: vw_sb[:, (kc * 4 + m) * 128:][:, :128]
            pw_l = lambda r, m: pw_sb[:, (r * 3 + m) * 128:][:, :128]
            vld_l = lambda r, t: vld_sb[:, (r * 9 + t) * 128:][:, :128]

            # Single-shot build: issue the first super-group's x loads ahead
            # of the (serial) constant loads so they overlap on the DMA
            # engines. For_i timing builds keep the stationary in-body load.
            preload_x = None
            if reps == 1:
                preload_x = [
                    xp.tile([128, SG * NP], f16, name=f"x_sb{kc}", tag=f"x{kc}")
                    for kc in range(3)
                ]
                for kc in range(3):
                    nc.sync.dma_start(
                        out=preload_x[kc], in_=x_t.ap()[kc, :, 0:SG, :]
                    )

            # q2p: (128, SG*4*128) f16: col block (ee*4+p)*128 holds pair p's
            # zero-padded lhsT: rows 32p:32p+32 = q data (even head cols 0:49,
            # odd cols 64:113), all other rows zero -> a full-128 contraction
            # against k_sb picks out exactly pair p (no PE retiling needed).
            q2p = consts.tile([128, SG * 4 * 128], f16, name="q2p")
            nc.gpsimd.memset(q2p[:, :], 0.0)

            from contextlib import nullcontext

            # ---- per-super-group emission, software-pipelined one SG deep:
            # AV/vld/proj of SG n-1 is emitted AFTER attention of SG n, so its
            # PE work fills the Act/DVE-bound softmax chain.
            state = {"x_pref": preload_x}

            def emit_sg_front(sg):
                e0 = sg * SG
                # ---- x for this super-group (host-padded 16x16) ----
                if state["x_pref"] is None or not use_prefetch:
                    x_cur = [
                        xp.tile([128, SG * NP], f16, name=f"x_sb{kc}", tag=f"x{kc}")
                        for kc in range(3)
                    ]
                    for kc in range(3):
                        nc.sync.dma_start(
                            out=x_cur[kc], in_=x_t.ap()[kc, :, e0 : e0 + SG, :]
                        )
                else:
                    x_cur = state["x_pref"]
                state["x_pref"] = None
                xr = [
                    x_cur[kc].rearrange("p (e h w) -> p e h w", e=SG, h=HP)
                    for kc in range(3)
                ]

                # ================= Q path =================
                ps_q = psC.tile([128, SG * 49], f32, name="ps_q", tag="c")
                # t==5 (depthwise center tap) reads the same strided view as
                # the pool tap t==0, so its weights are folded into slot 0.
                for t in range(10):
                    if t == 5:
                        continue
                    if t == 0:  # pool + center: padded (1::2, 1::2)
                        rv = lambda kc: xr[kc][:, :, 1:14:2, 1:14:2]
                    else:
                        dy, dx = TAPS[t - 1]
                        rv = lambda kc, dy=dy, dx=dx: xr[kc][
                            :, :, dy : dy + 13 : 2, dx : dx + 13 : 2
                        ]
                    for kc in range(3):
                        nc.tensor.matmul(
                            ps_q, qw_l(kc, t), rv(kc),
                            start=(t == 0 and kc == 0),
                            stop=(t == 9 and kc == 2),
                            skip_group_check=True,
                        )
                q_sb = sbs.tile([128, SG * 49], f16, name="q_sb", tag="q")
                nc.scalar.activation(
                    out=q_sb, in_=ps_q, func=AF.Identity,
                    bias=bvec_sb[:, 12:13], scale=1.0,
                )
                # build q2p (zero-interleaved pair lhsT blocks) on DVE
                q2pv = q2p.rearrange("p (e g r) -> p e g r", e=SG, g=4)
                q_sbv = q_sb.rearrange("p (e c) -> p e c", e=SG)
                for p in range(4):
                    nc.vector.tensor_copy(
                        out=q2pv[32 * p : 32 * p + 16, :, p, 0:49],
                        in_=q_sbv[32 * p : 32 * p + 16, :, :],
                    )
                    if q2p_pool:
                        nc.gpsimd.tensor_copy(
                            out=q2pv[32 * p + 16 : 32 * p + 32, :, p, 64:113],
                            in_=q_sbv[32 * p + 16 : 32 * p + 32, :, :],
                        )
                    else:
                        nc.sync.dma_start(
                            out=q2pv[32 * p + 16 : 32 * p + 32, :, p, 64:113],
                            in_=q_sbv[32 * p + 16 : 32 * p + 32, :, :],
                        )

                # ---- prefetch next super-group's x (keeps SP ahead) ----
                if use_prefetch and sg + 1 < nsg:
                    x_pref = [
                        xp.tile([128, SG * NP], f16, name=f"x_sb{kc}", tag=f"x{kc}")
                        for kc in range(3)
                    ]
                    for kc in range(3):
                        nc.sync.dma_start(
                            out=x_pref[kc],
                            in_=x_t.ap()[kc, :, e0 + SG : e0 + 2 * SG, :],
                        )
                    state["x_pref"] = x_pref

                # ================= K convs (pairs) ======================
                k_sb = sbkv.tile([128, SG * NP], f16, name="k_sb", tag="k")
                kp = k_sb.rearrange("p (e h w) -> p e h w", e=SG, h=HP)
                nc.gpsimd.memset(kp[:, :, 0:16:15, :], 0.0)
                nc.gpsimd.memset(kp[:, :, 1:15, 0:16:15], 0.0)
                for p2 in range(SG // 2):
                    es = p2 * 2
                    xin = [xr[kc][:, es : es + 2, 1:15, 1:15] for kc in range(3)]
                    ps_k = psC.tile([128, 2 * N], f32, name="ps_k", tag="c")
                    for kc in range(3):
                        nc.tensor.matmul(
                            ps_k, kw_l(kc), xin[kc],
                            start=(kc == 0), stop=(kc == 2),
                        )
                    nc.vector.tensor_scalar_add(
                        out=kp[:, es : es + 2, 1:15, 1:15], in0=ps_k,
                        scalar1=bvec_sb[:, 11:12],
                    )

                # ================= V convs (pairs) ======================
                # v4all: (128, SG, 4, 256) channel groups m on free dim
                v4all = sbkv.tile([128, SG * 4 * NP], f16, name="v4all", tag="v4")
                v4v = v4all.rearrange("p (e m h w) -> p e m h w", e=SG, m=4, h=HP)
                for m in range(4):
                    nc.gpsimd.memset(v4v[:, :, m, 0:16:15, :], 0.0)
                    nc.gpsimd.memset(v4v[:, :, m, 1:15, 0:16:15], 0.0)
                for p2 in range(SG // 2):
                    es = p2 * 2
                    xin = [xr[kc][:, es : es + 2, 1:15, 1:15] for kc in range(3)]
                    for m in range(4):
                        ps_v = psC.tile([128, 2 * N], f32, name="ps_v", tag="c")
                        for kc in range(3):
                            nc.tensor.matmul(
                                ps_v, vw_l(kc, m), xin[kc],
                                start=(kc == 0), stop=(kc == 2),
                            )
                        nc.vector.tensor_scalar_add(
                            out=v4v[:, es : es + 2, m, 1:15, 1:15], in0=ps_v,
                            scalar1=bvec_sb[:, m : m + 1],
                        )

                # ========= attention: S -> exp -> renorm; XBAR transposes
                at_t = [None] * SG
                v4t_t = [None] * SG
                a2_t = [None] * SG

                def emit_at(ee):
                    at = sbat.tile([128, 8 * 128], f16, name="at", tag=f"at{ee}")
                    if use_xbar:
                        nc.sync.dma_start_transpose(
                            out=at.rearrange("p (b c) -> p b c", b=8),
                            in_=a2_t[ee][:, :],
                        )
                    else:
                        psta = psT.tile([128, 1024], f16, name="psta", tag="ta")
                        for blk in range(8):
                            nc.tensor.transpose(
                                psta[:, 128 * blk :][:, :128],
                                a2_t[ee][:, 128 * blk :][:, :128],
                                iden_sb,
                            )
                        nc.vector.tensor_copy(out=at, in_=psta)
                    at_t[ee] = at

                # V^T XBAR transposes for all examples up-front: they only
                # need the V convs, so SP streams them while PE runs S/exp.
                def emit_vt(ee):
                    v4t = sbvt.tile([128, 8 * 128], f16, name="v4t", tag=f"v4t{ee}")
                    if use_xbar:
                        nc.sync.dma_start_transpose(
                            out=v4t.rearrange("p (b c) -> p b c", b=8),
                            in_=v4all[:, ee * 4 * NP :][:, : 4 * NP],
                        )
                    else:
                        pstv = psT.tile([128, 1024], f16, name="pstv", tag="tv")
                        for blk in range(8):
                            nc.tensor.transpose(
                                pstv[:, 128 * blk :][:, :128],
                                v4all[:, ee * 4 * NP + 128 * blk :][:, :128],
                                iden_sb,
                            )
                        nc.vector.tensor_copy(out=v4t, in_=pstv)
                    v4t_t[ee] = v4t

                if hoist_vt:
                    for ee in range(SG):
                        emit_vt(ee)

                for ee in range(SG):
                    if not hoist_vt:
                        emit_vt(ee)
                    # S: bias matmul (start) + one matmul per head pair
                    ps2 = [
                        psS.tile([128, 2 * NP], f32, name=f"ps2_{j}", tag=f"s{j}")
                        for j in range(2)
                    ]
                    for j in range(2):
                        if bias_dma:
                            nc.gpsimd.dma_start(
                                out=ps2[j][:, :],
                                in_=biasp32_sb[:, 512 * j :][:, :512],
                            )
                        else:
                            nc.tensor.matmul(
                                ps2[j], iden_sb, biasp_sb[:, 512 * j :][:, :512],
                                start=True, stop=False, skip_group_check=True,
                            )
                        for b in range(2):
                            p = 2 * j + b
                            nc.tensor.matmul(
                                ps2[j][:, NP * b :][:, :NP],
                                q2p[:, (ee * 4 + p) * 128 :][:, :128],
                                k_sb[:, ee * NP :][:, :NP],
                                start=False, stop=(b == 1),
                                skip_group_check=True,
                            )
                    # exp + per-pair denominators, then renorm
                    den = sbs.tile([128, 4], f32, name="den", tag="den")
                    a2 = sba.tile([128, 4 * NP], f16, name="a2", tag="a2")
                    a2_t[ee] = a2
                    for p in range(4):
                        nc.scalar.activation(
                            out=a2[:, NP * p :][:, :NP],
                            in_=ps2[p // 2][:, NP * (p % 2) :][:, :NP],
                            func=AF.Exp,
                            accum_out=den[:, p : p + 1],
                        )
                    rden = sbs.tile([128, 4], f32, name="rden", tag="rden")
                    nc.vector.reciprocal(out=rden, in_=den)
                    for p in range(4):
                        sl = a2[:, NP * p :][:, :NP]
                        nc.vector.tensor_scalar_mul(
                            out=sl, in0=sl, scalar1=rden[:, p : p + 1]
                        )
                    if ee >= at_depth:
                        emit_at(ee - at_depth)
                for ee in range(SG - at_depth, SG):
                    emit_at(ee)
                return {"e0": e0, "v4v": v4v, "at_t": at_t, "v4t_t": v4t_t}

            def emit_sg_back(ctx):
                e0, v4v, at_t, v4t_t = (
                    ctx["e0"], ctx["v4v"], ctx["at_t"], ctx["v4t_t"]
                )
                # ==== v_local (diag matmuls) + AV accumulate + relu ====
                z_sb = []
                for r in range(4):
                    ps_vl = psVL.tile([128, SG * 49], f32, name="ps_vl", tag="vl")
                    for ti, (dy, dx) in enumerate(TAPS_C):
                        nc.tensor.matmul(
                            ps_vl,
                            vld_l(r, TAPS.index((dy, dx))),
                            v4v[:, :, r, dy : dy + 13 : 2, dx : dx + 13 : 2],
                            start=(ti == 0), stop=False, skip_group_check=True,
                        )
                    for ee in range(SG):
                        for hh in range(2):
                            out_sl = ps_vl[
                                64 * hh : 64 * hh + 64, ee * 49 : (ee + 1) * 49
                            ]
                            for half in range(2):
                                co = 128 * (2 * r + half) + 64 * hh
                                nc.tensor.matmul(
                                    out_sl,
                                    v4t_t[ee][:, co : co + 64],
                                    at_t[ee][:, co : co + 49],
                                    start=False,
                                    stop=(ee == SG - 1 and hh == 1 and half == 1),
                                    skip_group_check=True,
                                )
                    zz = sbz.tile([128, SG * 49], f16, name="z_sb", tag=f"z{r}")
                    nc.vector.tensor_scalar(
                        out=zz, in0=ps_vl,
                        scalar1=bvec_sb[:, 7 + r : 8 + r], scalar2=0.0,
                        op0=mybir.AluOpType.add, op1=mybir.AluOpType.max,
                    )
                    z_sb.append(zz)

                # ================= projection + store =================
                for m in range(3):
                    ps_y = psC.tile([128, SG * 49], f32, name="ps_y", tag="c")
                    for r in range(4):
                        nc.tensor.matmul(
                            ps_y, pw_l(r, m), z_sb[r],
                            start=(r == 0), stop=(r == 3),
                        )
                    yy = sby.tile([128, SG * 49], f32, name="y_sb", tag="y")
                    nc.vector.tensor_scalar_add(
                        out=yy, in0=ps_y, scalar1=bvec_sb[:, 4 + m : 5 + m],
                    )
                    # y stores issue from the Act engine's DGE: keeps the SP
                    # stream free so the next super-group's x loads issue
                    # before the projection tail completes.
                    eng = {"sp": nc.sync, "act": nc.scalar,
                           "pool": nc.gpsimd}[y_eng]
                    eng.dma_start(out=y_t.ap()[m, :, e0 : e0 + SG, :], in_=yy)

            loop_cm = tc.For_i(0, reps, 1) if reps > 1 else nullcontext()
            with loop_cm:
                pend = None
                for sg in range(nsg):
                    ctx = emit_sg_front(sg)
                    if pend is not None:
                        emit_sg_back(pend)
                    pend = ctx
                emit_sg_back(pend)
    nc.compile()
    return nc


def prep_weights(inputs):
    """Host-side: fold BN/scales, build device weight layouts."""
    f = lambda a: np.asarray(a, np.float32)
    scale = KD ** -0.5

    kw2 = f(inputs["k_w"])[:, :, 0, 0] * f(inputs["k_bn_s"])[:, None]  # (128,384)
    kb2 = f(inputs["k_b"]) * f(inputs["k_bn_s"]) + f(inputs["k_bn_b"])
    qw2 = f(inputs["q_proj_w"])[:, :, 0, 0] * f(inputs["q_bn_s"])[:, None] * scale
    qb2 = (
        f(inputs["q_proj_b"]) * f(inputs["q_bn_s"]) + f(inputs["q_bn_b"])
    ) * scale + qw2 @ f(inputs["q_local_b"])
    qlw = f(inputs["q_local_w"])[:, 0].reshape(C, 9)
    vw2 = f(inputs["v_w"])[:, :, 0, 0] * f(inputs["v_bn_s"])[:, None]  # (512,384)
    vb2 = f(inputs["v_b"]) * f(inputs["v_bn_s"]) + f(inputs["v_bn_b"])
    vlw = f(inputs["vl_w"])[:, 0].reshape(DH, 9) * f(inputs["vl_bn_s"])[:, None]
    vlb = f(inputs["vl_b"]) * f(inputs["vl_bn_s"]) + f(inputs["vl_bn_b"])
    pw2 = f(inputs["p_w"])[:, :, 0, 0] * f(inputs["p_bn_s"])[:, None]  # (384,512)
    pb2 = f(inputs["p_b"]) * f(inputs["p_bn_s"]) + f(inputs["p_bn_b"])

    qw_arr = np.zeros((128, 3, 10, 128), np.float32)
    for kc in range(3):
        cs = slice(128 * kc, 128 * kc + 128)
        qw_arr[:, kc, 0, :] = qw2[:, cs].T
        for t in range(9):
            qw_arr[:, kc, 1 + t, :] = qw2[:, cs].T * qlw[cs, t][:, None]
        # center tap (1,1) = TAPS[4] -> slot 5 reads the same view as the
        # pool tap; fold it into slot 0 and skip slot 5 on device.
        qw_arr[:, kc, 0, :] += qw_arr[:, kc, 5, :]
    kw_arr = np.zeros((128, 3, 128), np.float32)
    for kc in range(3):
        kw_arr[:, kc, :] = kw2[:, 128 * kc : 128 * kc + 128].T
    vw_arr = np.zeros((128, 3, 4, 128), np.float32)
    for kc in range(3):
        for m in range(4):
            vw_arr[:, kc, m, :] = vw2[128 * m : 128 * m + 128, 128 * kc : 128 * kc + 128].T
    pw_arr = np.zeros((128, 4, 3, 128), np.float32)
    for r in range(4):
        for m in range(3):
            pw_arr[:, r, m, :] = pw2[128 * m : 128 * m + 128, 128 * r : 128 * r + 128].T
    vld_arr = np.zeros((128, 4, 9, 128), np.float32)
    ii = np.arange(128)
    for r in range(4):
        for t in range(9):
            vld_arr[ii, r, t, ii] = vlw[128 * r : 128 * r + 128, t]

    # biasp: (128, 1024): [r, 512j+256b+kk] = bias(pair 2j+b, row r, key kk)
    # rows r<49: even head q=r; 64<=r<113: odd head q=r-64; else 0.
    # halo key cols get -60000 on data rows so exp -> 0.
    bt = f(inputs["bias_tab"]).reshape(HEADS, N2, 14, 14)
    btp = np.full((HEADS, N2, HP, WP), -60000.0, np.float32)
    btp[:, :, 1:15, 1:15] = bt
    btp = btp.reshape(HEADS, N2, NP)
    biasp = np.zeros((128, 1024), np.float32)
    for p in range(4):
        col = 256 * p
        biasp[0:49, col : col + NP] = btp[2 * p]
        biasp[64:113, col : col + NP] = btp[2 * p + 1]

    bvec = np.zeros((128, 13), np.float32)
    for m in range(4):
        bvec[:, m] = vb2[128 * m : 128 * m + 128]
    for m in range(3):
        bvec[:, 4 + m] = pb2[128 * m : 128 * m + 128]
    for r in range(4):
        bvec[:, 7 + r] = vlb[128 * r : 128 * r + 128]
    bvec[:, 11] = kb2
    bvec[:, 12] = qb2

    return {
        "biasp32": biasp.astype(np.float32),
        "qw": qw_arr.astype(np.float16),
        "kw": kw_arr.astype(np.float16),
        "vw": vw_arr.astype(np.float16),
        "pw": pw_arr.astype(np.float16),
        "vld": vld_arr.astype(np.float16),
        "biasp": biasp.astype(np.float16),
        "iden": np.eye(128, dtype=np.float16),
        "bvec": bvec,
    }


def prep_x_core(x, c, epc=EPC):
    """x (B, C, H, W) -> per-core (3, 128, epc, 256) fp16 with zero halo."""
    xc = np.asarray(x, np.float32)[c * epc : (c + 1) * epc]  # (epc, C, 14, 14)
    xp = np.zeros((epc, C, HP, WP), np.float32)
    xp[:, :, 1:15, 1:15] = xc
    xp = xp.reshape(epc, C, NP).transpose(1, 0, 2).reshape(3, 128, epc, NP)
    return xp.astype(np.float16)


def unpack_y(y, epc=EPC):
    """(3, 128, epc, 49) fp32 -> (epc, 384, 7, 7)."""
    return (
        np.asarray(y, np.float32)
        .reshape(OUT, epc, 49)
        .transpose(1, 0, 2)
        .reshape(epc, OUT, 7, 7)
    )


_CACHE = {}


def kernel(**inputs) -> np.ndarray:
    from concourse import bass_utils

    if "nc" not in _CACHE:
        _CACHE["nc"] = build_bass()
    nc = _CACHE["nc"]

    wmaps = prep_weights(inputs)
    in_maps = []
    for c in range(NCORES):
        m = dict(wmaps)
        m["x"] = prep_x_core(inputs["x"], c)
        in_maps.append(m)

    res = bass_utils.run_bass_kernel_spmd(nc, in_maps, core_ids=list(range(NCORES)))
    outs = [unpack_y(r["y"]) for r in res.results]
    return np.concatenate(outs, axis=0)


if __name__ == "__main__":
    print("building bass program...")
    nc = build_bass()
    print("build OK")
